# revision 9
# baseline (speedup 1.0000x reference)
"""AttentionWithContext on 8 NeuronCores (Trainium2, Bass/Tile).

Sharding: batch x head-group. Core (b, g) with b in 0..3, g in 0..1 computes
batch b, heads g*8..g*8+8 (Megatron column-parallel QKV, row-parallel proj).
Host pre-transposes/slices weights per core (cast to fp16 host-side), sums
the two partial proj outputs per batch and adds the bias.

Per-core pipeline (fp16 matmul inputs, fp32 PSUM accumulate), designed to
keep the PE continuously busy (p-state at max clock):
  upfront: q+k projections for pair 0 (PSUM accumulate over 8 C-chunks)
  per pair p, head h, kv-chunk mc (M=1280 = 2 ctx chunks + 8 x chunks):
    QK^T:   s[mc] = kT^T q  (S^T layout [kv, n], 2x [128,512] mm)
    exp:    eST[mc] = exp(scale*s)  (ACT, fp16 out)
    AV:     av[nt] += v_aug[:,h,mc]^T eST[mc]  emitted ONE chunk late so the
            PE never waits on ACT; v_aug carries 64 ones-columns so av rows
            64..127 accumulate the softmax denominator l replicated 64x
    filler: vproj / next pair q,k proj / prev pair out-proj between chunks
  norm:   OT = av[0:64] * reciprocal(av[64:128]) straight from PSUM (fp16)
  proj:   y_acc[n, co] (+)= OT[ci,n]^T w2T[ci,co], DVE accumulate in SBUF
PSUM: s_ps [128,1024]x2 + av [128,512]x2 + filler [128,512]x2 = 8 banks.

Shapes (fixed): x (4,1024,1024), context (4,256,2048), w_qkv (3072,1024),
w_proj (1024,1024), b_proj (1024,). H=16 heads, D=64, N=1024, N_c=256.
"""
import sys

if "/opt/trn_rl_repo" not in sys.path:
    sys.path.insert(0, "/opt/trn_rl_repo")

from collections import deque

import numpy as np

import concourse.bass as bass
import concourse.mybir as mybir
import concourse.tile as tile
from concourse import bacc, bass_utils

B, N, C = 4, 1024, 1024
H, D = 16, 64
NC_ = 256            # context length
M = NC_ + N          # kv length = 1280
HG = 8               # heads per core
NCORES = 8
SCALE = D ** -0.5
CCH = C // 128       # 8 contraction chunks
MCH = M // 128       # 10 kv chunks
NCH = N // 128       # 8 query chunks

f32 = mybir.dt.float32
DT = mybir.dt.float16
NPDT = np.float16
AF = mybir.ActivationFunctionType

_compiled = None


def _build():
    nc = bacc.Bacc("TRN2", target_bir_lowering=False, debug=False,
                   num_devices=NCORES)
    xT_ap = nc.dram_tensor("xT", [C, N], DT, kind="ExternalInput").ap()
    wqk_ap = nc.dram_tensor("wqkT", [C, 4, 256], DT, kind="ExternalInput").ap()
    wv_ap = nc.dram_tensor("wvT", [C, 512], DT, kind="ExternalInput").ap()
    ctxk_ap = nc.dram_tensor("ctxkT", [4, 128, NC_], DT, kind="ExternalInput").ap()
    ctxv_ap = nc.dram_tensor("ctxv2", [128, HG, 2, D], DT, kind="ExternalInput").ap()
    w2_ap = nc.dram_tensor("w2T", [512, C], DT, kind="ExternalInput").ap()
    y_ap = nc.dram_tensor("y", [N, C], f32, kind="ExternalOutput").ap()

    with tile.TileContext(nc) as tc:
        with (
            tc.tile_pool(name="sb", bufs=1) as sb,
            tc.tile_pool(name="ps", bufs=1, space="PSUM") as ps,
        ):
            # ---- SBUF tiles ----
            xTr = sb.tile([128, CCH, N], DT, tag="r_x")
            wqkr = sb.tile([128, CCH, 4, 256], DT, tag="r_wqk")
            wvTr = sb.tile([128, CCH, 512], DT, tag="r_wv")
            w2Tr = sb.tile([128, 4, C], DT, tag="r_w2")
            # v_aug: cols 0:64 ones (denominator -> av rows 0:64, partition
            # offset 0 for reciprocal_approx_fast), cols 64:128 per-head v
            v_aug = sb.tile([128, HG, MCH, 128], DT, tag="vaug")
            kT2 = [sb.tile([128, M], DT, tag=f"kT2_{p}", name=f"kT2_{p}")
                   for p in range(4)]
            qT2 = [sb.tile([128, N], DT, tag=f"qT2_{p}", name=f"qT2_{p}")
                   for p in range(4)]
            OT = sb.tile([128, 4, C], DT, tag="OT")
            y_acc = sb.tile([128, NCH, C], f32, tag="yacc")
            scr_i = sb.tile([1, 16], f32, tag="scr_i")
            scr_o = sb.tile([1, 16], DT, tag="scr_o")

            # ---- t=0: act exp-table preload; v_aug ones on DVE (its queue
            # is idle early and nothing waits on its counter until ~11us) ----
            nc.gpsimd.memset(scr_i[:], 0.5)
            nc.scalar.activation(scr_o[:], scr_i[:], AF.Exp, scale=1.0)
            nc.vector.memset(v_aug[:, :, :, 0:D], 1.0)

            # ---- input DMA (fp16 straight from DRAM), spread on 3 queues;
            # first-needed tensors lead each queue ----
            xT_src = xT_ap.rearrange("(c p) n -> p c n", p=128)
            wqk_src = wqk_ap.rearrange("(c p) q j -> p c q j", p=128)
            ctxk_src = ctxk_ap.rearrange("q p m -> p q m")
            qsy, qgp, qsc = nc.sync, nc.gpsimd, nc.scalar
            qsy.dma_start(xTr[:, 0, :], xT_src[:, 0, :])
            qgp.dma_start(wqkr[:, :, 0, :], wqk_src[:, :, 0, :])
            qsy.dma_start(xTr[:, 1:3, :], xT_src[:, 1:3, :])
            qgp.dma_start(xTr[:, 3:5, :], xT_src[:, 3:5, :])
            qsy.dma_start(xTr[:, 5:8, :], xT_src[:, 5:8, :])
            for mcx in range(2):
                qsc.dma_start(v_aug[:, :, mcx, D:128], ctxv_ap[:, :, mcx, :])
            for p in range(4):
                qgp.dma_start(kT2[p][:, 0:NC_], ctxk_src[:, p, :])
            qsy.dma_start(wvTr[:], wv_ap.rearrange("(c p) j -> p c j", p=128))
            qgp.dma_start(wqkr[:, :, 1, :], wqk_src[:, :, 1, :])
            yq = [qsy, qgp, qsc]

            # ---- PE work generators (interleaved into attention) ----
            def emit_qkproj(p):
                """q,k projections for pair p (4 fat pulls)."""
                for kind in range(2):
                    for nh in range(2):
                        acc = ps.tile([128, 512], f32, tag="b512", bufs=2,
                                      name=f"qk_ps{p}_{kind}{nh}")
                        for cc in range(CCH):
                            nc.tensor.matmul(
                                acc[:],
                                wqkr[:, cc, p, kind * 128:(kind + 1) * 128],
                                xTr[:, cc, nh * 512:(nh + 1) * 512],
                                start=(cc == 0), stop=(cc == CCH - 1),
                            )
                        if kind == 0:
                            nc.vector.tensor_copy(
                                qT2[p][:, nh * 512:(nh + 1) * 512], acc[:])
                        else:
                            nc.vector.tensor_copy(
                                kT2[p][:, NC_ + nh * 512:NC_ + (nh + 1) * 512],
                                acc[:])
                        yield

            def emit_vproj():
                """v for all heads; fills v_aug kv-chunks 2..9 (8 fat pulls)."""
                for nch in range(NCH):
                    acc = ps.tile([128, 512], f32, tag="b512", bufs=2,
                                  name=f"v_ps{nch}")
                    for cc in range(CCH):
                        nc.tensor.matmul(
                            acc[:],
                            xTr[:, cc, nch * 128:(nch + 1) * 128],
                            wvTr[:, cc, :],
                            start=(cc == 0), stop=(cc == CCH - 1),
                        )
                    nc.vector.tensor_copy(
                        v_aug[:, :, nch + 2, D:128],
                        acc[:].rearrange("p (h d) -> p h d", d=D))
                    yield

            def emit_projpair(p):
                """Pair p's contribution to y (16 thin pulls)."""
                for nch in range(NCH):
                    for cot in range(2):
                        t = ps.tile([128, 512], f32, tag="b512", bufs=2,
                                    name=f"y_ps{p}_{nch}{cot}")
                        nc.tensor.matmul(
                            t[:],
                            OT[:, p, nch * 128:(nch + 1) * 128],
                            w2Tr[:, p, cot * 512:(cot + 1) * 512],
                            start=True, stop=True,
                        )
                        dst = y_acc[:, nch, cot * 512:(cot + 1) * 512]
                        if p == 0:
                            nc.vector.tensor_copy(dst, t[:])
                        else:
                            nc.vector.tensor_add(dst, dst, t[:])
                        yield

            def emit_projpair3():
                """Pair 3 runs after all attention: the s_ps pool is free, so
                use [128,1024] tiles (both cot halves) and one wide DVE add
                per n-chunk to shorten the drain."""
                for nch in range(NCH):
                    t = ps.tile([128, N], f32, tag="qk1024", bufs=2,
                                name=f"y3_ps{nch}")
                    for cot in range(2):
                        nc.tensor.matmul(
                            t[:, cot * 512:(cot + 1) * 512],
                            OT[:, 3, nch * 128:(nch + 1) * 128],
                            w2Tr[:, 3, cot * 512:(cot + 1) * 512],
                            start=True, stop=True,
                        )
                    dst = y_acc[:, nch, :]
                    nc.vector.tensor_add(dst, dst, t[:])
                    yq[nch % 3].dma_start(
                        y_ap[nch * 128:(nch + 1) * 128, :], dst)
                    yield

            def emit_attention(p, filler, pulls=1):
                """Attention for heads 2p, 2p+1; AV lags QK by one kv-chunk
                so the PE never waits on ACT's exp."""
                for hh in range(2):
                    h = 2 * p + hh
                    hb = hh * 64
                    av = [ps.tile([128, 512], f32, tag="av", bufs=2,
                                  name=f"av{h}_{nt}") for nt in range(2)]

                    def emit_av(mc, eST):
                        for nt in range(2):
                            nc.tensor.matmul(
                                av[nt][:],
                                v_aug[:, h, mc, :],
                                eST[:, nt * 512:(nt + 1) * 512],
                                start=(mc == 0), stop=(mc == MCH - 1),
                            )

                    prev = None
                    for mc in range(MCH):
                        s = ps.tile([128, N], f32, tag="qk1024", bufs=2,
                                    name=f"s_ps{h}_{mc}")
                        for nt in range(2):
                            nc.tensor.matmul(
                                s[:, nt * 512:(nt + 1) * 512],
                                kT2[p][hb:hb + 64, mc * 128:(mc + 1) * 128],
                                qT2[p][hb:hb + 64, nt * 512:(nt + 1) * 512],
                                start=True, stop=True,
                            )
                        eST = sb.tile([128, N], DT, tag="eST", bufs=4,
                                      name=f"eST{h}_{mc}")
                        nc.scalar.activation(eST[:], s[:], AF.Exp,
                                             scale=float(SCALE))
                        for _ in range(pulls):
                            next(filler)
                        if prev is not None:
                            emit_av(*prev)
                        prev = (mc, eST)
                    emit_av(*prev)
                    # normalize straight from PSUM: OT = av[64:128] / l
                    # (l in av rows 0:64 - reciprocal_approx_fast drops the
                    # partition offset of its input AP, so l must sit at 0)
                    for nt in range(2):
                        linv = sb.tile([64, 512], f32, tag="linv", bufs=2,
                                       name=f"linv{h}_{nt}")
                        nc.vector.reciprocal_approx_fast(
                            linv[:], av[nt][0:64, :])
                        nc.vector.tensor_mul(
                            OT[hb:hb + 64, p, nt * 512:(nt + 1) * 512],
                            av[nt][64:128, :], linv[:])

            # ---- emission schedule ----
            fq = deque([emit_vproj(), emit_qkproj(1)])

            class _Filler:
                def __next__(self):
                    while fq:
                        try:
                            next(fq[0])
                            return
                        except StopIteration:
                            fq.popleft()
            filler = _Filler()

            for _ in emit_qkproj(0):
                pass
            emit_attention(0, filler)
            qsy.dma_start(wqkr[:, :, 2, :], wqk_src[:, :, 2, :])
            qgp.dma_start(wqkr[:, :, 3, :], wqk_src[:, :, 3, :])
            qsy.dma_start(w2Tr[:], w2_ap.rearrange("(c p) j -> p c j", p=128))
            fq.append(emit_qkproj(2))
            fq.append(emit_projpair(0))
            emit_attention(1, filler)
            fq.append(emit_qkproj(3))
            fq.append(emit_projpair(1))
            emit_attention(2, filler)
            fq.append(emit_projpair(2))
            emit_attention(3, filler)
            fq.append(emit_projpair3())
            while fq:
                try:
                    next(fq[0])
                except StopIteration:
                    fq.popleft()

    nc.compile()
    return nc


def _get_compiled():
    global _compiled
    if _compiled is None:
        _compiled = _build()
    return _compiled


def _prep_core_inputs(x, context, w_qkv, w_proj):
    """Build the per-core input maps (numpy, host-side sharding + fp16)."""
    in_maps = []
    for core in range(NCORES):
        b, g = core // 2, core % 2
        h0 = g * HG
        xT = np.ascontiguousarray(x[b].T, dtype=NPDT)           # [C, N]
        q_rows = w_qkv[h0 * D:(h0 + HG) * D]                    # [512, C]
        k_rows = w_qkv[C + h0 * D:C + (h0 + HG) * D]
        v_rows = w_qkv[2 * C + h0 * D:2 * C + (h0 + HG) * D]
        # [C, 4 pairs, 256 = q(128) | k(128)]
        wqkT = np.ascontiguousarray(
            np.concatenate([q_rows.reshape(4, 128, C),
                            k_rows.reshape(4, 128, C)], axis=1)
            .transpose(2, 0, 1), dtype=NPDT)
        wvT = np.ascontiguousarray(v_rows.T, dtype=NPDT)        # [C, 512]
        ctx = context[b].reshape(NC_, 2, H, D)
        ctx_k = ctx[:, 0, h0:h0 + HG, :]                        # [256, 8, 64]
        ctx_v = ctx[:, 1, h0:h0 + HG, :]
        # ctxkT: [4 pairs, 128 = 2 heads x 64 d, 256 m]
        ctxkT = np.ascontiguousarray(
            ctx_k.transpose(1, 2, 0).reshape(4, 128, NC_), dtype=NPDT)
        # ctxv2: [128 p, 8 h, 2 mc, 64 d]; kv position = mc*128 + p
        ctxv2 = np.ascontiguousarray(
            ctx_v.reshape(2, 128, HG, D).transpose(1, 2, 0, 3), dtype=NPDT)
        w2T = np.ascontiguousarray(w_proj[:, h0 * D:(h0 + HG) * D].T,
                                   dtype=NPDT)                  # [512, C]
        in_maps.append({
            "xT": xT, "wqkT": wqkT, "wvT": wvT,
            "ctxkT": ctxkT, "ctxv2": ctxv2, "w2T": w2T,
        })
    return in_maps


def kernel(x, context, w_qkv, w_proj, b_proj, _trace=False):
    x = np.asarray(x, dtype=np.float32)
    context = np.asarray(context, dtype=np.float32)
    w_qkv = np.asarray(w_qkv, dtype=np.float32)
    w_proj = np.asarray(w_proj, dtype=np.float32)
    b_proj = np.asarray(b_proj, dtype=np.float32)

    nc = _get_compiled()
    in_maps = _prep_core_inputs(x, context, w_qkv, w_proj)
    res = bass_utils.run_bass_kernel_spmd(
        nc, in_maps, list(range(NCORES)), trace=_trace)
    kernel.last_results = res

    out = np.empty((B, N, C), np.float32)
    for b in range(B):
        out[b] = (res.results[2 * b]["y"] + res.results[2 * b + 1]["y"]
                  + b_proj)
    return out


# revision 10
# speedup vs baseline: 1.1707x; 1.1707x over previous
"""AttentionWithContext on 8 NeuronCores (Trainium2, Bass/Tile).

Sharding: batch x head-group. Core (b, g) with b in 0..3, g in 0..1 computes
batch b, heads g*8..g*8+8 (Megatron column-parallel QKV, row-parallel proj).
Host pre-transposes/slices weights per core (cast to fp16 host-side), sums
the two partial proj outputs per batch and adds the bias.

Per-core pipeline (fp16 matmul inputs, fp32 PSUM accumulate), designed to
keep the PE continuously busy (p-state at max clock):
  upfront: q+k projections for pair 0 (PSUM accumulate over 8 C-chunks)
  per pair p, head h, kv-chunk mc (M=1280 = 2 ctx chunks + 8 x chunks):
    QK^T:   s[mc] = kT^T q  (S^T layout [kv, n], 2x [128,512] mm)
    exp:    eST[mc] = exp(scale*s)  (ACT, fp16 out)
    AV:     av[nt] += v_aug[:,h,mc]^T eST[mc]  emitted ONE chunk late so the
            PE never waits on ACT; v_aug carries 64 ones-columns so av rows
            64..127 accumulate the softmax denominator l replicated 64x
    filler: vproj / next pair q,k proj / prev pair out-proj between chunks
  norm:   OT = av[0:64] * reciprocal(av[64:128]) straight from PSUM (fp16)
  proj:   y_acc[n, co] (+)= OT[ci,n]^T w2T[ci,co], DVE accumulate in SBUF
PSUM: s_ps [128,1024]x2 + av [128,512]x2 + filler [128,512]x2 = 8 banks.

Shapes (fixed): x (4,1024,1024), context (4,256,2048), w_qkv (3072,1024),
w_proj (1024,1024), b_proj (1024,). H=16 heads, D=64, N=1024, N_c=256.
"""
import sys

if "/opt/trn_rl_repo" not in sys.path:
    sys.path.insert(0, "/opt/trn_rl_repo")

from collections import deque

import numpy as np

import concourse.bass as bass
import concourse.mybir as mybir
import concourse.tile as tile
from concourse import bacc, bass_utils

B, N, C = 4, 1024, 1024
H, D = 16, 64
NC_ = 256            # context length
M = NC_ + N          # kv length = 1280
HG = 8               # heads per core
NCORES = 8
SCALE = D ** -0.5
CCH = C // 128       # 8 contraction chunks
MCH = M // 128       # 10 kv chunks
NCH = N // 128       # 8 query chunks

f32 = mybir.dt.float32
DT = mybir.dt.float16
NPDT = np.float16
AF = mybir.ActivationFunctionType

_compiled = None


def _build():
    nc = bacc.Bacc("TRN2", target_bir_lowering=False, debug=False,
                   num_devices=NCORES)
    xT_ap = nc.dram_tensor("xT", [C, N], DT, kind="ExternalInput").ap()
    wqk_ap = nc.dram_tensor("wqkT", [C, 4, 256], DT, kind="ExternalInput").ap()
    wv_ap = nc.dram_tensor("wvT", [C, 512], DT, kind="ExternalInput").ap()
    ctxk_ap = nc.dram_tensor("ctxkT", [4, 128, NC_], DT, kind="ExternalInput").ap()
    ctxv_ap = nc.dram_tensor("ctxv2", [128, HG, 2, D], DT, kind="ExternalInput").ap()
    w2_ap = nc.dram_tensor("w2T", [512, C], DT, kind="ExternalInput").ap()
    y_ap = nc.dram_tensor("y", [N, C], DT, kind="ExternalOutput").ap()

    with tile.TileContext(nc) as tc:
        with (
            tc.tile_pool(name="sb", bufs=1) as sb,
            tc.tile_pool(name="ps", bufs=1, space="PSUM") as ps,
        ):
            # ---- SBUF tiles ----
            xTr = sb.tile([128, CCH, N], DT, tag="r_x")
            wqkr = sb.tile([128, CCH, 4, 256], DT, tag="r_wqk")
            wvTr = sb.tile([128, CCH, 512], DT, tag="r_wv")
            w2Tr = sb.tile([128, 4, C], DT, tag="r_w2")
            # v_aug: cols 0:64 ones (denominator -> av rows 0:64, partition
            # offset 0 for reciprocal_approx_fast), cols 64:128 per-head v
            v_aug = sb.tile([128, HG, MCH, 128], DT, tag="vaug")
            kT2 = [sb.tile([128, M], DT, tag=f"kT2_{p}", name=f"kT2_{p}")
                   for p in range(4)]
            qT2 = [sb.tile([128, N], DT, tag=f"qT2_{p}", name=f"qT2_{p}")
                   for p in range(4)]
            OT = sb.tile([128, 4, C], DT, tag="OT")
            y_acc = sb.tile([128, NCH, C], f32, tag="yacc")
            y16 = sb.tile([128, NCH, C], DT, tag="y16")
            scr_i = sb.tile([1, 16], f32, tag="scr_i")
            scr_o = sb.tile([1, 16], DT, tag="scr_o")

            # ---- t=0: act exp-table preload; v_aug ones on DVE (its queue
            # is idle early and nothing waits on its counter until ~11us) ----
            nc.gpsimd.memset(scr_i[:], 0.5)
            nc.scalar.activation(scr_o[:], scr_i[:], AF.Exp, scale=1.0)
            nc.vector.memset(v_aug[:, :, :, 0:D], 1.0)

            # ---- input DMA (fp16 straight from DRAM), spread on 3 queues;
            # first-needed tensors lead each queue ----
            xT_src = xT_ap.rearrange("(c p) n -> p c n", p=128)
            wqk_src = wqk_ap.rearrange("(c p) q j -> p c q j", p=128)
            ctxk_src = ctxk_ap.rearrange("q p m -> p q m")
            qsy, qgp, qsc = nc.sync, nc.gpsimd, nc.scalar
            qsy.dma_start(xTr[:, 0, :], xT_src[:, 0, :])
            qgp.dma_start(wqkr[:, :, 0, :], wqk_src[:, :, 0, :])
            qsy.dma_start(xTr[:, 1:3, :], xT_src[:, 1:3, :])
            qgp.dma_start(xTr[:, 3:5, :], xT_src[:, 3:5, :])
            qsy.dma_start(xTr[:, 5:8, :], xT_src[:, 5:8, :])
            yq = [qsy, qgp, qsc]

            def emit_late_dmas():
                """Everything not needed by qkproj(0): emitted after it so
                the upfront matmuls' per-queue DMA-counter waits don't
                include these transfers."""
                for mcx in range(2):
                    qsc.dma_start(v_aug[:, :, mcx, D:128],
                                  ctxv_ap[:, :, mcx, :])
                for p in range(4):
                    qgp.dma_start(kT2[p][:, 0:NC_], ctxk_src[:, p, :])
                qsy.dma_start(wvTr[:],
                              wv_ap.rearrange("(c p) j -> p c j", p=128))
                qgp.dma_start(wqkr[:, :, 1, :], wqk_src[:, :, 1, :])

            # ---- PE work generators (interleaved into attention) ----
            def emit_qkproj(p):
                """q,k projections for pair p (4 fat pulls)."""
                for kind in range(2):
                    for nh in range(2):
                        acc = ps.tile([128, 512], f32, tag="b512", bufs=2,
                                      name=f"qk_ps{p}_{kind}{nh}")
                        for cc in range(CCH):
                            nc.tensor.matmul(
                                acc[:],
                                wqkr[:, cc, p, kind * 128:(kind + 1) * 128],
                                xTr[:, cc, nh * 512:(nh + 1) * 512],
                                start=(cc == 0), stop=(cc == CCH - 1),
                            )
                        if kind == 0:
                            nc.vector.tensor_copy(
                                qT2[p][:, nh * 512:(nh + 1) * 512], acc[:])
                        else:
                            nc.vector.tensor_copy(
                                kT2[p][:, NC_ + nh * 512:NC_ + (nh + 1) * 512],
                                acc[:])
                        yield

            def emit_vproj():
                """v for all heads; fills v_aug kv-chunks 2..9 (8 fat pulls)."""
                for nch in range(NCH):
                    acc = ps.tile([128, 512], f32, tag="b512", bufs=2,
                                  name=f"v_ps{nch}")
                    for cc in range(CCH):
                        nc.tensor.matmul(
                            acc[:],
                            xTr[:, cc, nch * 128:(nch + 1) * 128],
                            wvTr[:, cc, :],
                            start=(cc == 0), stop=(cc == CCH - 1),
                        )
                    nc.vector.tensor_copy(
                        v_aug[:, :, nch + 2, D:128],
                        acc[:].rearrange("p (h d) -> p h d", d=D))
                    yield

            def emit_projpair(p):
                """Pair p's contribution to y (16 thin pulls)."""
                for nch in range(NCH):
                    for cot in range(2):
                        t = ps.tile([128, 512], f32, tag="b512", bufs=2,
                                    name=f"y_ps{p}_{nch}{cot}")
                        nc.tensor.matmul(
                            t[:],
                            OT[:, p, nch * 128:(nch + 1) * 128],
                            w2Tr[:, p, cot * 512:(cot + 1) * 512],
                            start=True, stop=True,
                        )
                        dst = y_acc[:, nch, cot * 512:(cot + 1) * 512]
                        if p == 0:
                            nc.vector.tensor_copy(dst, t[:])
                        else:
                            nc.vector.tensor_add(dst, dst, t[:])
                        yield

            def emit_projpair3():
                """Pair 3 runs after all attention: the s_ps pool is free, so
                use [128,1024] tiles (both cot halves) and one wide DVE add
                per n-chunk to shorten the drain."""
                for nch in range(NCH):
                    t = ps.tile([128, N], f32, tag="qk1024", bufs=2,
                                name=f"y3_ps{nch}")
                    for cot in range(2):
                        nc.tensor.matmul(
                            t[:, cot * 512:(cot + 1) * 512],
                            OT[:, 3, nch * 128:(nch + 1) * 128],
                            w2Tr[:, 3, cot * 512:(cot + 1) * 512],
                            start=True, stop=True,
                        )
                    dst = y16[:, nch, :]
                    nc.vector.tensor_add(dst, y_acc[:, nch, :], t[:])
                    yq[nch % 3].dma_start(
                        y_ap[nch * 128:(nch + 1) * 128, :], dst)
                    yield

            def emit_attention(p, filler, pulls=1):
                """Attention for heads 2p, 2p+1; AV lags QK by one kv-chunk
                so the PE never waits on ACT's exp."""
                for hh in range(2):
                    h = 2 * p + hh
                    hb = hh * 64
                    av = [ps.tile([128, 512], f32, tag="av", bufs=2,
                                  name=f"av{h}_{nt}") for nt in range(2)]

                    def emit_av(mc, eST):
                        for nt in range(2):
                            nc.tensor.matmul(
                                av[nt][:],
                                v_aug[:, h, mc, :],
                                eST[:, nt * 512:(nt + 1) * 512],
                                start=(mc == 0), stop=(mc == MCH - 1),
                            )

                    prev = None
                    for mc in range(MCH):
                        s = ps.tile([128, N], f32, tag="qk1024", bufs=2,
                                    name=f"s_ps{h}_{mc}")
                        for nt in range(2):
                            nc.tensor.matmul(
                                s[:, nt * 512:(nt + 1) * 512],
                                kT2[p][hb:hb + 64, mc * 128:(mc + 1) * 128],
                                qT2[p][hb:hb + 64, nt * 512:(nt + 1) * 512],
                                start=True, stop=True,
                            )
                        eST = sb.tile([128, N], DT, tag="eST", bufs=4,
                                      name=f"eST{h}_{mc}")
                        nc.scalar.activation(eST[:], s[:], AF.Exp,
                                             scale=float(SCALE))
                        for _ in range(pulls):
                            next(filler)
                        if prev is not None:
                            emit_av(*prev)
                        prev = (mc, eST)
                    emit_av(*prev)
                    # normalize straight from PSUM: OT = av[64:128] / l
                    # (l in av rows 0:64 - reciprocal_approx_fast drops the
                    # partition offset of its input AP, so l must sit at 0)
                    for nt in range(2):
                        linv = sb.tile([64, 512], f32, tag="linv", bufs=2,
                                       name=f"linv{h}_{nt}")
                        nc.vector.reciprocal_approx_fast(
                            linv[:], av[nt][0:64, :])
                        nc.vector.tensor_mul(
                            OT[hb:hb + 64, p, nt * 512:(nt + 1) * 512],
                            av[nt][64:128, :], linv[:])

            # ---- emission schedule ----
            fq = deque([emit_vproj(), emit_qkproj(1)])

            class _Filler:
                def __next__(self):
                    while fq:
                        try:
                            next(fq[0])
                            return
                        except StopIteration:
                            fq.popleft()
            filler = _Filler()

            qk0 = emit_qkproj(0)
            next(qk0)
            emit_late_dmas()
            for _ in qk0:
                pass
            emit_attention(0, filler)
            qsy.dma_start(wqkr[:, :, 2, :], wqk_src[:, :, 2, :])
            qgp.dma_start(wqkr[:, :, 3, :], wqk_src[:, :, 3, :])
            qsy.dma_start(w2Tr[:], w2_ap.rearrange("(c p) j -> p c j", p=128))
            fq.append(emit_qkproj(2))
            fq.append(emit_projpair(0))
            emit_attention(1, filler)
            fq.append(emit_qkproj(3))
            fq.append(emit_projpair(1))
            emit_attention(2, filler)
            fq.append(emit_projpair(2))
            emit_attention(3, filler)
            fq.append(emit_projpair3())
            while fq:
                try:
                    next(fq[0])
                except StopIteration:
                    fq.popleft()

    nc.compile()
    return nc


def _get_compiled():
    global _compiled
    if _compiled is None:
        _compiled = _build()
    return _compiled


def _prep_core_inputs(x, context, w_qkv, w_proj):
    """Build the per-core input maps (numpy, host-side sharding + fp16)."""
    in_maps = []
    for core in range(NCORES):
        b, g = core // 2, core % 2
        h0 = g * HG
        xT = np.ascontiguousarray(x[b].T, dtype=NPDT)           # [C, N]
        q_rows = w_qkv[h0 * D:(h0 + HG) * D]                    # [512, C]
        k_rows = w_qkv[C + h0 * D:C + (h0 + HG) * D]
        v_rows = w_qkv[2 * C + h0 * D:2 * C + (h0 + HG) * D]
        # [C, 4 pairs, 256 = q(128) | k(128)]
        wqkT = np.ascontiguousarray(
            np.concatenate([q_rows.reshape(4, 128, C),
                            k_rows.reshape(4, 128, C)], axis=1)
            .transpose(2, 0, 1), dtype=NPDT)
        wvT = np.ascontiguousarray(v_rows.T, dtype=NPDT)        # [C, 512]
        ctx = context[b].reshape(NC_, 2, H, D)
        ctx_k = ctx[:, 0, h0:h0 + HG, :]                        # [256, 8, 64]
        ctx_v = ctx[:, 1, h0:h0 + HG, :]
        # ctxkT: [4 pairs, 128 = 2 heads x 64 d, 256 m]
        ctxkT = np.ascontiguousarray(
            ctx_k.transpose(1, 2, 0).reshape(4, 128, NC_), dtype=NPDT)
        # ctxv2: [128 p, 8 h, 2 mc, 64 d]; kv position = mc*128 + p
        ctxv2 = np.ascontiguousarray(
            ctx_v.reshape(2, 128, HG, D).transpose(1, 2, 0, 3), dtype=NPDT)
        w2T = np.ascontiguousarray(w_proj[:, h0 * D:(h0 + HG) * D].T,
                                   dtype=NPDT)                  # [512, C]
        in_maps.append({
            "xT": xT, "wqkT": wqkT, "wvT": wvT,
            "ctxkT": ctxkT, "ctxv2": ctxv2, "w2T": w2T,
        })
    return in_maps


def kernel(x, context, w_qkv, w_proj, b_proj, _trace=False):
    x = np.asarray(x, dtype=np.float32)
    context = np.asarray(context, dtype=np.float32)
    w_qkv = np.asarray(w_qkv, dtype=np.float32)
    w_proj = np.asarray(w_proj, dtype=np.float32)
    b_proj = np.asarray(b_proj, dtype=np.float32)

    nc = _get_compiled()
    in_maps = _prep_core_inputs(x, context, w_qkv, w_proj)
    res = bass_utils.run_bass_kernel_spmd(
        nc, in_maps, list(range(NCORES)), trace=_trace)
    kernel.last_results = res

    out = np.empty((B, N, C), np.float32)
    for b in range(B):
        out[b] = (res.results[2 * b]["y"].astype(np.float32)
                  + res.results[2 * b + 1]["y"].astype(np.float32) + b_proj)
    return out
